# revision 37
# baseline (speedup 1.0000x reference)
"""Trainium2 Bass kernel for nn_Attention_6992206758310.

Dense transformer block: LayerNorm -> QKV -> selective RoPE -> head-last
masked attention (softmax over j) -> out-projection.

Sharding: heads (16) are split 2-per-core across 8 NeuronCores (tensor
parallel). LayerNorm is REPLICATED on every core (it is cheap and runs
entirely inside the ~66us collective-entry dead window, eliminating the
AllGather a sharded LayerNorm would need). Each core computes QKV + RoPE
for its 2 heads over the full sequence, runs attention in sim^T [j, i]
layout (softmax over the partition axis becomes a matmul-accumulated
column sum via an appended ones-column on V), normalizes by the softmax
denominator on the head-parallel side, AllToAll-reshards to
sequence-parallel, and projects through w_out so each core emits its own
256-row slice of the output. Host concatenates slices.

All matmuls are bf16 (fp32 PSUM accumulate). The scalar engine runs only
Exp (plus one batched Rsqrt + Identity copies) to avoid activation-table
thrash; other elementwise work is on vector/gpsimd. Dummy fp32 matmuls
during the AllToAll keep the PE clock-gate warm for the out-projection.
"""
import numpy as np

N_SEQ = 2048
DIM = 1024
H = 16
DH = 64
NC = 8
HPC = 2           # heads per core
CW = HPC * DH     # 128 local head-dim columns
ISL = N_SEQ // NC # 256 output rows per core
LN_EPS = 1e-6
NEG = -1e30

_CACHE = {}


def _av_segments(off):
    """Column segments of a 1024-wide block, split at PSUM bank (512) bounds."""
    if off < 512:
        return [(off, 512), (512, 1024)]
    return [(off, 1024)]


def _build(debug=False):
    import concourse.bass as bass
    import concourse.bacc as bacc
    import concourse.tile as tile
    import concourse.mybir as mybir

    F32 = mybir.dt.float32
    BF = mybir.dt.bfloat16
    AF = mybir.ActivationFunctionType
    ALU = mybir.AluOpType

    nc = bacc.Bacc("TRN2", target_bir_lowering=False, debug=False, num_devices=NC)

    x_d = nc.dram_tensor("x_bf", [N_SEQ, DIM], BF, kind="ExternalInput")
    wblk_d = nc.dram_tensor("w_blk", [DIM, 3 * CW], BF, kind="ExternalInput")
    wout_d = nc.dram_tensor("w_out", [DIM, DIM], BF, kind="ExternalInput")
    qb_d = nc.dram_tensor("qb", [128, 3], F32, kind="ExternalInput")
    cos_d = nc.dram_tensor("cos2t", [CW, N_SEQ], BF, kind="ExternalInput")
    sin_d = nc.dram_tensor("sin2t", [CW, N_SEQ], BF, kind="ExternalInput")
    pb_d = nc.dram_tensor("pb2d", [128, 16], F32, kind="ExternalInput")
    pb01_d = nc.dram_tensor("pb01", [128, 16], BF, kind="ExternalInput")
    tri_d = nc.dram_tensor("tri2", [128, 256], BF, kind="ExternalInput")
    p128_d = nc.dram_tensor("p128", [128, 128], BF, kind="ExternalInput")
    ident_d = nc.dram_tensor("ident", [128, 128], BF, kind="ExternalInput")
    out_d = nc.dram_tensor("out_sl", [ISL, DIM], F32, kind="ExternalOutput")

    groups = [list(range(NC))]
    KC = DIM // 128  # 8 contraction chunks
    NB = N_SEQ // 128  # 16 sequence blocks

    with tile.TileContext(nc) as tc:
        with tc.tile_pool(name="cst", bufs=1) as cst, \
             tc.tile_pool(name="big", bufs=1) as big, \
             tc.tile_pool(name="wrk", bufs=2) as wrk, \
             tc.tile_pool(name="xb", bufs=1) as xbp, \
             tc.tile_pool(name="et", bufs=6) as etp, \
             tc.tile_pool(name="dram", bufs=1, space="DRAM") as drp:

            a2a_in = drp.tile([NC * 128, ISL], BF, tag="a2a_in")
            a2a_out = drp.tile([NC * 128, ISL], BF, tag="a2a_out")
            wup_in = drp.tile([128, 8], BF, tag="wup_in")
            wup_out = drp.tile([NC * 128, 8], BF, tag="wup_out", addr_space="Shared")

            # ---------- warm-up collective enqueued first ----------
            # absorbs the collective-entry barrier during LayerNorm and keeps
            # the ncfw stream stepping so the real A2A later starts fast.
            wup_sb = cst.tile([128, 8], BF, tag="wup_sb")
            nc.vector.memset(wup_sb[:], 0.0)
            nc.sync.dma_start(wup_in[:, :], wup_sb[:])
            nc.gpsimd.collective_compute(
                "AllGather", ALU.bypass, replica_groups=groups,
                ins=[wup_in.opt()], outs=[wup_out.opt()])

            # ---------- inputs: x split across 4 DMA queues, consts behind ----------
            x_all = xbp.tile([128, NB, DIM], BF, tag="x_all")
            xsrc = x_d.ap().rearrange("(b p) d -> p b d", p=128)
            dma_engs = [nc.sync, nc.scalar]
            for b in range(NB):
                dma_engs[b % 2].dma_start(
                    x_all[:, b:b + 1, :], xsrc[:, b:b + 1, :])
            xblk = [x_all[:, b, :] for b in range(NB)]
            ident_t = cst.tile([128, 128], BF, tag="ident")
            nc.sync.dma_start(ident_t[:], ident_d.ap())
            qb_t = cst.tile([128, 3], F32, tag="qb")
            nc.sync.dma_start(qb_t[:], qb_d.ap())
            pb_t = cst.tile([128, 16], F32, tag="pb")
            pb01_t = cst.tile([128, 16], BF, tag="pb01")
            tri_t = cst.tile([128, 256], BF, tag="tri")
            p128_t = cst.tile([128, 128], BF, tag="p128")
            nc.sync.dma_start(pb_t[:], pb_d.ap())
            nc.sync.dma_start(pb01_t[:], pb01_d.ap())
            nc.sync.dma_start(tri_t[:], tri_d.ap())
            nc.sync.dma_start(p128_t[:], p128_d.ap())
            w_all = cst.tile([128, KC, 3 * CW], BF, tag="w_all")
            nc.sync.dma_start(
                w_all[:], wblk_d.ap().rearrange("(kc p) c -> p kc c", p=128))
            w_t = [w_all[:, kc, :] for kc in range(KC)]
            cos_t = cst.tile([CW, N_SEQ], BF, tag="cos")
            sin_t = cst.tile([CW, N_SEQ], BF, tag="sin")
            nc.scalar.dma_start(cos_t[:], cos_d.ap())
            nc.scalar.dma_start(sin_t[:], sin_d.ap())
            wo_all = cst.tile([128, KC, DIM], BF, tag="wo_all")
            nc.sync.dma_start(
                wo_all[:], wout_d.ap().rearrange("(kc p) c -> p kc c", p=128))
            wo_t = [wo_all[:, kc, :] for kc in range(KC)]

            zeps = cst.tile([128, 2], F32, tag="zeps")
            nc.vector.memset(zeps[:, 0:1], 0.0)
            nc.vector.memset(zeps[:, 1:2], LN_EPS)
            ones64 = cst.tile([1, 64], BF, tag="ones64")
            nc.vector.memset(ones64[:], 1.0)
            # av lhsT per j-chunk: [v_h0(64) | 1 | v_h1(64) | 1] -> 130 cols each
            av_all = big.tile([128, 16 * 130], BF, tag="av_all")
            av_v = av_all[:].rearrange("p (jc c) -> p jc c", c=130)
            nc.vector.memset(av_v[:, :, 64:65], 1.0)
            nc.vector.memset(av_v[:, :, 129:130], 1.0)

            psC = tc.tile_pool(name="psC", bufs=1, space="PSUM")
            ps = psC.__enter__()

            # ---------- phase 1: replicated LayerNorm ----------
            # stats on DVE (bn_stats/bn_aggr), batched Rsqrt on scalar,
            # normalize-apply on gpsimd, transpose copies split scalar/DVE.
            mv_all = wrk.tile([128, 2 * NB], F32, tag="mv")
            for b in range(NB):
                st = wrk.tile([128, 12], F32, tag="st6", bufs=4)
                nc.vector.bn_stats(st[:, 0:6], xblk[b][:, 0:512])
                nc.vector.bn_stats(st[:, 6:12], xblk[b][:, 512:1024])
                nc.vector.bn_aggr(mv_all[:, 2 * b:2 * b + 2], st[:])
            rstd_all = wrk.tile([128, NB], F32, tag="rstd")
            nmr_all = wrk.tile([128, NB], F32, tag="nmr")
            for g in range(8):
                gs = slice(g * 2, (g + 1) * 2)
                # rstd = 1/sqrt(var + eps)
                sqv = wrk.tile([128, 2], F32, tag="sqv", bufs=4)
                nc.scalar.activation(
                    sqv[:],
                    mv_all[:].rearrange("p (b tw) -> p b tw", tw=2)[:, gs, 1:2],
                    AF.Sqrt, bias=zeps[:, 1:2])
                nc.vector.reciprocal(rstd_all[:, gs], sqv[:])
                # nmr = -mean * rstd
                nc.vector.scalar_tensor_tensor(
                    nmr_all[:, gs],
                    mv_all[:].rearrange("p (b tw) -> p b tw", tw=2)[:, gs, 0:1],
                    -1.0, rstd_all[:, gs], ALU.mult, ALU.mult)
            xnT_g = []
            for grp in range(2):
                t = big.tile([128, NB * 512], BF, tag=f"xnTg{grp}")
                xnT_g.append(t)
            for b in range(NB):
                xn = wrk.tile([128, DIM], BF, tag="ln_xn", bufs=4)
                nc.gpsimd.tensor_scalar(
                    xn[:], xblk[b],
                    rstd_all[:, b:b + 1], nmr_all[:, b:b + 1],
                    ALU.mult, ALU.add)
                for grp in range(2):
                    tp_f = ps.tile([128, 1024], F32, tag="sim", bufs=2)
                    tp = tp_f[:].bitcast(BF)
                    for q in range(4):
                        kc = grp * 4 + q
                        nc.tensor.transpose(
                            tp[:, q * 128:(q + 1) * 128],
                            xn[:, kc * 128:(kc + 1) * 128], ident_t[:])
                    dst = xnT_g[grp][:, b * 512:(b + 1) * 512]
                    nc.scalar.copy(dst, tp[:, 0:512])

            # ---------- phase 2: qkv^T (weight-stationary halves) + rope ----------
            # Emission order on PE: all QKV matmuls (both halves), then the
            # rotation matmuls (their qT/kT inputs were copied out long
            # before), then V transposes - so the PE never waits on copies.
            qropeT = big.tile([CW, N_SEQ], BF, tag="qropeT")
            krop_h0 = big.tile([CW, N_SEQ], BF, tag="krop_h0")
            krop_h1 = big.tile([CW, N_SEQ], BF, tag="krop_h1")
            nc.vector.memset(krop_h0[64:128, :], 0.0)
            nc.vector.memset(krop_h1[0:64, :], 0.0)
            kroph = [krop_h0, krop_h1]
            vT_sb = big.tile([CW, N_SEQ], BF, tag="vT")
            qT_h = [None, None]
            kT_h = [None, None]

            def emit_qkv(half, cp_vec):
                hc = slice(half * 1024, (half + 1) * 1024)
                ps_q = ps.tile([128, 1024], F32, tag="sim", bufs=2)
                ps_k = ps.tile([128, 1024], F32, tag="sim", bufs=2)
                ps_v = ps.tile([128, 1024], F32, tag="sim", bufs=2)
                for kc in range(KC):
                    st = (kc == 0); sp = (kc == KC - 1)
                    grp, kcq = kc // 4, kc % 4
                    xview = xnT_g[grp][:].rearrange(
                        "p (b q c) -> p b q c", q=4, c=128)
                    for seg in range(2):
                        cs = slice(seg * 512, (seg + 1) * 512)
                        b0 = 8 * half + 4 * seg
                        rhs = xview[:, b0:b0 + 4, kcq, :]
                        nc.tensor.matmul(ps_q[:, cs], w_t[kc][:, 0:128], rhs,
                                         start=st, stop=sp, skip_group_check=True)
                        nc.tensor.matmul(ps_k[:, cs], w_t[kc][:, 128:256], rhs,
                                         start=st, stop=sp, skip_group_check=True)
                        nc.tensor.matmul(ps_v[:, cs], w_t[kc][:, 256:384], rhs,
                                         start=st, stop=sp, skip_group_check=True)
                qT_sb = wrk.tile([128, 1024], BF, tag="qT_sb")
                kT_sb = wrk.tile([128, 1024], BF, tag="kT_sb")
                if cp_vec:
                    nc.vector.tensor_scalar_add(qT_sb[:], ps_q[:], qb_t[:, 0:1])
                    nc.vector.tensor_scalar_add(kT_sb[:], ps_k[:], qb_t[:, 1:2])
                    nc.vector.tensor_scalar_add(vT_sb[:, hc], ps_v[:], qb_t[:, 2:3])
                else:
                    nc.scalar.activation(qT_sb[:], ps_q[:], AF.Identity, bias=qb_t[:, 0:1])
                    nc.scalar.activation(kT_sb[:], ps_k[:], AF.Identity, bias=qb_t[:, 1:2])
                    nc.scalar.activation(vT_sb[:, hc], ps_v[:], AF.Identity, bias=qb_t[:, 2:3])
                qT_h[half] = qT_sb
                kT_h[half] = kT_sb

            def emit_rope(half):
                hc = slice(half * 1024, (half + 1) * 1024)
                ps_qr = ps.tile([128, 1024], F32, tag="sim", bufs=2)
                ps_kr = ps.tile([128, 1024], F32, tag="sim", bufs=2)
                for seg in range(2):
                    cs = slice(seg * 512, (seg + 1) * 512)
                    nc.tensor.matmul(ps_qr[:, cs], p128_t[:], qT_h[half][:, cs],
                                     start=True, stop=True, skip_group_check=True)
                    nc.tensor.matmul(ps_kr[:, cs], p128_t[:], kT_h[half][:, cs],
                                     start=True, stop=True, skip_group_check=True)
                for ki, (src_sb, src_r) in enumerate(((qT_h[half], ps_qr),
                                                      (kT_h[half], ps_kr))):
                    t1 = wrk.tile([128, 1024], BF, tag="rp1")
                    nc.gpsimd.tensor_mul(t1[:], src_sb[:], cos_t[:, hc])
                    t2 = wrk.tile([128, 1024], BF, tag="rp2")
                    nc.vector.scalar_tensor_tensor(
                        t2[:], src_r[:], 1.0, sin_t[:, hc], ALU.mult, ALU.mult)
                    if ki == 0:
                        nc.vector.tensor_add(qropeT[:, hc], t1[:], t2[:])
                    else:
                        nc.vector.tensor_add(
                            krop_h0[0:64, hc], t1[0:64, :], t2[0:64, :])
                        nc.vector.tensor_add(
                            krop_h1[64:128, hc], t1[64:128, :], t2[64:128, :])

            def emit_vt(half):
                for grp in range(2 * half, 2 * half + 2):
                    tp_f = ps.tile([128, 1024], F32, tag="sim", bufs=2)
                    tp = tp_f[:].bitcast(BF)
                    for q in range(4):
                        jc = grp * 4 + q
                        nc.tensor.transpose(
                            tp[:, q * 128:(q + 1) * 128],
                            vT_sb[:, jc * 128:(jc + 1) * 128], ident_t[:])
                    for q in range(4):
                        jc = grp * 4 + q
                        src = tp[:, q * 128:(q + 1) * 128]
                        eng = nc.scalar.copy if q % 2 == 0 else (
                            lambda d, s: nc.vector.tensor_scalar_mul(d, s, 1.0))
                        eng(av_all[:, jc * 130 + 0: jc * 130 + 64], src[:, 0:64])
                        eng(av_all[:, jc * 130 + 65: jc * 130 + 129], src[:, 64:128])

            # ---------- attention + pre-A2A normalize (interleaved with
            # half-1 QKV so the scalar-bound exp stream starts early) ----------
            avh_all = {}

            def emit_sim(ib4, jc):
                i0 = ib4 * 1024
                off = max(0, 128 * jc - i0)
                diag = 128 * jc >= i0
                tsel = 0 if jc == 0 else 128
                ets = []
                for h in range(2):
                    sim = ps.tile([128, 1024], F32, tag="sim", bufs=2)
                    for (a, b) in _av_segments(off):
                        has_diag = diag and a <= off < b
                        nc.tensor.matmul(
                            sim[:, a:b],
                            kroph[h][:, jc * 128:(jc + 1) * 128],
                            qropeT[:, i0 + a:i0 + b],
                            start=True, stop=not has_diag,
                            skip_group_check=True)
                        if has_diag:
                            # causal mask added on the PE itself
                            nc.tensor.matmul(
                                sim[:, off:off + 128],
                                ident_t[:], tri_t[:, tsel:tsel + 128],
                                start=False, stop=True,
                                skip_group_check=True)
                    e_t = etp.tile([128, 1024], BF, tag="e_t")
                    nc.scalar.activation(e_t[:, off:], sim[:, off:], AF.Exp,
                                         bias=pb_t[:, jc:jc + 1])
                    ets.append(e_t)
                return ets

            def emit_av(ib4, jc, ets):
                i0 = ib4 * 1024
                jmax = 8 * ib4 + 7
                off = max(0, 128 * jc - i0)
                for h in range(2):
                    for (a, b) in _av_segments(off):
                        last = (ib4 == 1 and jc == jmax and b == 1024)
                        nc.tensor.matmul(
                            avh_all[(ib4, h)][:, a:b],
                            av_all[:, jc * 130 + 65 * h: jc * 130 + 65 * h + 65],
                            ets[h][:, a:b],
                            start=(jc == 0), stop=last,
                            skip_group_check=True)

            def attn_main(ib4):
                for h in range(2):
                    av_t = ps.tile([65, 1024], F32, tag="av", bufs=2)
                    avh_all[(ib4, h)] = av_t
                jmax = 8 * ib4 + 7
                pend = None
                for jc in range(jmax + 1):
                    ets = emit_sim(ib4, jc)
                    if pend is not None:
                        emit_av(ib4, pend[0], pend[1])
                    pend = (jc, ets)
                emit_av(ib4, pend[0], pend[1])

            def attn_tail(ib4):
                for h in range(2):
                    av = avh_all[(ib4, h)]
                    if ib4 == 0:
                        # column i=0 attends to all j: chunks 1..15 add col 0
                        e0full = ps.tile([128, 1024], F32, tag="sim", bufs=2)
                        e0ps = e0full[:, 0:16]
                        for jc in range(1, 16):
                            nc.tensor.matmul(
                                e0ps[:, jc:jc + 1],
                                kroph[h][:, jc * 128:(jc + 1) * 128],
                                qropeT[:, 0:1],
                                start=(jc == 1), stop=(jc == 15), skip_group_check=True)
                        e0e = wrk.tile([128, 16], BF, tag="e0e")
                        nc.scalar.activation(e0e[:], e0ps[:], AF.Exp, bias=zeps[:, 0:1])
                        e0m = wrk.tile([128, 16], BF, tag="e0m")
                        nc.vector.tensor_mul(e0m[:], e0e[:], pb01_t[:])
                        for jc in range(1, 16):
                            nc.tensor.matmul(
                                av[:, 0:1],
                                av_all[:, jc * 130 + 65 * h: jc * 130 + 65 * h + 65],
                                e0m[:, jc:jc + 1],
                                start=False, stop=(jc == 15), skip_group_check=True)
                    # normalize: avn = av[0:64] / av[64]
                    den = wrk.tile([1, 1024], BF, tag="den")
                    nc.vector.tensor_scalar_mul(den[:], av[64:65, :], 1.0)
                    bps = ps.tile([128, 1024], F32, tag="sim", bufs=2)
                    for seg in range(2):
                        cs = slice(seg * 512, (seg + 1) * 512)
                        nc.tensor.matmul(bps[0:64, cs], ones64[:], den[:, cs],
                                         start=True, stop=True, skip_group_check=True)
                    recb = wrk.tile([64, 1024], F32, tag="recb")
                    nc.vector.reciprocal_approx_fast(recb[:], bps[0:64, :])
                    avs = wrk.tile([64, 1024], BF, tag="avs")
                    nc.vector.tensor_mul(avs[:], av[0:64, :], recb[:])
                    nc.sync.dma_start(
                        a2a_in[:].rearrange("(blk p) i -> p blk i", p=128)
                               [64 * h:64 * h + 64, 4 * ib4:4 * ib4 + 4, :],
                        avs[:].rearrange("p (blk i) -> p blk i", blk=4))

            def emit_qkv1_piece(which):
                # one of q/k/v for half 1, fully accumulated + copied out so
                # the sim-tag rotation never stalls on a half-filled tile
                half = 1
                hc = slice(half * 1024, (half + 1) * 1024)
                pp = ps.tile([128, 1024], F32, tag="sim", bufs=2)
                col = {"q": 0, "k": 1, "v": 2}[which]
                for kc in range(KC):
                    st = (kc == 0); sp = (kc == KC - 1)
                    grp, kcq = kc // 4, kc % 4
                    xview = xnT_g[grp][:].rearrange(
                        "p (b q c) -> p b q c", q=4, c=128)
                    for seg in range(2):
                        cs = slice(seg * 512, (seg + 1) * 512)
                        b0 = 8 * half + 4 * seg
                        rhs = xview[:, b0:b0 + 4, kcq, :]
                        nc.tensor.matmul(
                            pp[:, cs], w_t[kc][:, col * 128:(col + 1) * 128],
                            rhs, start=st, stop=sp, skip_group_check=True)
                if which == "q":
                    qT_sb = wrk.tile([128, 1024], BF, tag="qT_sb")
                    nc.vector.tensor_scalar_add(qT_sb[:], pp[:], qb_t[:, 0:1])
                    qT_h[half] = qT_sb
                elif which == "k":
                    kT_sb = wrk.tile([128, 1024], BF, tag="kT_sb")
                    nc.vector.tensor_scalar_add(kT_sb[:], pp[:], qb_t[:, 1:2])
                    kT_h[half] = kT_sb
                else:
                    nc.vector.tensor_scalar_add(vT_sb[:, hc], pp[:], qb_t[:, 2:3])

            def attn_main0_interleaved():
                ib4 = 0
                for h in range(2):
                    av_t = ps.tile([65, 1024], F32, tag="av", bufs=2)
                    avh_all[(ib4, h)] = av_t
                pend = None
                for jc in range(8):
                    ets = emit_sim(ib4, jc)
                    if pend is not None:
                        emit_av(ib4, pend[0], pend[1])
                    pend = (jc, ets)
                    if jc == 2:
                        emit_qkv1_piece("q")
                    elif jc == 4:
                        emit_qkv1_piece("k")
                    elif jc == 6:
                        emit_qkv1_piece("v")
                emit_av(ib4, pend[0], pend[1])

            emit_qkv(0, cp_vec=False)
            emit_rope(0)
            emit_vt(0)
            attn_main0_interleaved()
            emit_rope(1)
            emit_vt(1)
            attn_tail(0)
            attn_main(1)
            attn_tail(1)

            # ---------- phase 5: A2A reshard heads -> sequence (bf16) ----------
            nc.gpsimd.collective_compute(
                "AllToAll", ALU.bypass, replica_groups=groups,
                ins=[a2a_in.opt()], outs=[a2a_out.opt()])

            psC.__exit__(None, None, None)
            psD = tc.tile_pool(name="psD", bufs=1, space="PSUM")
            ps = psD.__enter__()
            # ---------- keep the PE clock-gate warm through the A2A ----------
            dmy = ps.tile([128, 256], F32, tag="dmy")
            for _ in range(48):
                nc.tensor.matmul(dmy[:], tri_t[:, 0:128], tri_t[:, 0:256],
                                 start=True, stop=True, skip_group_check=True)
            # ---------- phase 6: out-projection ----------
            rcv_all = big.tile([128, NC * ISL], BF, tag="rcv_all")
            rcv_v = rcv_all[:].rearrange("p (blk i) -> p blk i", blk=NC)
            a2o_v = a2a_out[:].rearrange("(blk p) i -> p blk i", p=128)
            nc.sync.dma_start(rcv_v[:, 0:4, :], a2o_v[:, 0:4, :])
            nc.scalar.dma_start(rcv_v[:, 4:8, :], a2o_v[:, 4:8, :])
            for icx in range(2):
                op0 = ps.tile([128, 512], F32, tag="op", bufs=2)
                op1 = ps.tile([128, 512], F32, tag="op", bufs=2)
                for kb in range(NC):
                    st = (kb == 0); sp = (kb == NC - 1)
                    lhs = rcv_all[:, kb * ISL + icx * 128: kb * ISL + (icx + 1) * 128]
                    nc.tensor.matmul(op0[:], lhs, wo_t[kb][:, 0:512], start=st, stop=sp)
                    nc.tensor.matmul(op1[:], lhs, wo_t[kb][:, 512:1024], start=st, stop=sp)
                ob = wrk.tile([128, DIM], F32, tag="ob")
                nc.vector.tensor_scalar_mul(ob[:, 0:512], op0[:], 1.0)
                nc.vector.tensor_scalar_mul(ob[:, 512:1024], op1[:], 1.0)
                nc.sync.dma_start(out_d.ap()[icx * 128:(icx + 1) * 128, :], ob[:])
            psD.__exit__(None, None, None)

    nc.compile()
    return nc


def _host_prep(x, pos_sin, pos_cos, mask, ln_scale, ln_bias, w_qkv, w_out, b_out):
    f32 = np.float32
    import ml_dtypes
    bf16 = ml_dtypes.bfloat16
    scale = np.float32(DIM ** -0.5)
    x = np.asarray(x, f32); pos_sin = np.asarray(pos_sin, f32)
    pos_cos = np.asarray(pos_cos, f32); mask = np.asarray(mask)
    ln_scale = np.asarray(ln_scale, f32); ln_bias = np.asarray(ln_bias, f32)
    w_qkv = np.asarray(w_qkv, f32); w_out = np.asarray(w_out, f32)

    W = w_qkv * ln_scale[:, None]
    qb_full = (ln_bias @ w_qkv).astype(f32)  # [3072]

    x_bf = np.ascontiguousarray(x).astype(bf16)

    cos_full = np.ones((N_SEQ, DH // 2), f32)
    sin_full = np.zeros((N_SEQ, DH // 2), f32)
    cos_full[1:] = pos_cos
    sin_full[1:] = pos_sin
    cos2t = np.ascontiguousarray(np.tile(np.repeat(cos_full, 2, axis=1).T, (2, 1))).astype(bf16)
    sin2t = np.ascontiguousarray(np.tile(np.repeat(sin_full, 2, axis=1).T, (2, 1))).astype(bf16)

    pb_vec = np.zeros(N_SEQ, f32)
    pb_vec[1:] = np.where(mask, 0.0, NEG).astype(f32)
    pb2d = np.ascontiguousarray(pb_vec.reshape(16, 128).T)
    pb01 = np.ascontiguousarray((pb2d == 0)).astype(bf16)

    idg = np.arange(128)
    triu = (idg[None, :] >= idg[:, None])
    tri_first = np.where(triu | (idg[None, :] == 0), 0.0, NEG).astype(f32)
    tri_rest = np.where(triu, 0.0, NEG).astype(f32)
    tri2 = np.ascontiguousarray(np.concatenate([tri_first, tri_rest], axis=1)).astype(bf16)

    p128 = np.zeros((128, 128), f32)
    t = np.arange(64)
    p128[2 * t + 1, 2 * t] = -1.0
    p128[2 * t, 2 * t + 1] = 1.0
    p128 = p128.astype(bf16)

    ident = np.eye(128, dtype=f32).astype(bf16)
    w_out_c = np.ascontiguousarray(w_out).astype(bf16)

    in_maps = []
    for r in range(NC):
        hc = slice(CW * r, CW * (r + 1))
        w_blk = np.ascontiguousarray(np.concatenate(
            [W[:, 0:H * DH][:, hc] * scale,
             W[:, H * DH:2 * H * DH][:, hc],
             W[:, 2 * H * DH:][:, hc]], axis=1)).astype(bf16)
        qb = np.concatenate(
            [qb_full[0:H * DH][hc] * scale,
             qb_full[H * DH:2 * H * DH][hc],
             qb_full[2 * H * DH:][hc]]).astype(f32)
        in_maps.append({
            "x_bf": x_bf,
            "w_blk": w_blk,
            "w_out": w_out_c,
            "qb": np.ascontiguousarray(qb.reshape(3, CW).T),
            "cos2t": cos2t, "sin2t": sin2t,
            "pb2d": pb2d, "pb01": pb01, "tri2": tri2,
            "p128": p128, "ident": ident,
        })
    return in_maps


def _kernel_impl(inputs, trace=False, tmpdir=None):
    from concourse.bass_utils import run_bass_kernel_spmd
    if "nc" not in _CACHE:
        _CACHE["nc"] = _build()
    nc = _CACHE["nc"]
    in_maps = _host_prep(**inputs)
    kwargs = {}
    if trace:
        import sys, types
        try:
            from antenv.axon_hooks import get_axon_ntff_profile_hook  # noqa: F401
        except ImportError:
            from trn_agent_boot.trn_boot import _ntff_profile_via_ctypes
            hook = _ntff_profile_via_ctypes('/opt/axon/libaxon_pjrt.so')
            mod = types.ModuleType('antenv.axon_hooks')
            mod.get_axon_ntff_profile_hook = lambda: hook
            sys.modules['antenv.axon_hooks'] = mod
        kwargs = {"trace": True, "tmpdir": tmpdir}
    res = run_bass_kernel_spmd(nc, in_maps, list(range(NC)), **kwargs)
    out = np.concatenate([res.results[r]["out_sl"] for r in range(NC)], axis=0)
    out = out + np.asarray(inputs["b_out"], np.float32)[None, :]
    return out, res.exec_time_ns


def kernel(**inputs) -> np.ndarray:
    out, _ = _kernel_impl(inputs)
    return out


# revision 38
# speedup vs baseline: 1.0476x; 1.0476x over previous
"""Trainium2 Bass kernel for nn_Attention_6992206758310.

Dense transformer block: LayerNorm -> QKV -> selective RoPE -> head-last
masked attention (softmax over j) -> out-projection.

Sharding: heads (16) are split 2-per-core across 8 NeuronCores (tensor
parallel). LayerNorm is REPLICATED on every core (it is cheap and runs
entirely inside the ~66us collective-entry dead window, eliminating the
AllGather a sharded LayerNorm would need). Each core computes QKV + RoPE
for its 2 heads over the full sequence, runs attention in sim^T [j, i]
layout (softmax over the partition axis becomes a matmul-accumulated
column sum via an appended ones-column on V), normalizes by the softmax
denominator on the head-parallel side, AllToAll-reshards to
sequence-parallel, and projects through w_out so each core emits its own
256-row slice of the output. Host concatenates slices.

All matmuls are bf16 (fp32 PSUM accumulate). The scalar engine runs only
Exp (plus one batched Rsqrt + Identity copies) to avoid activation-table
thrash; other elementwise work is on vector/gpsimd. Dummy fp32 matmuls
during the AllToAll keep the PE clock-gate warm for the out-projection.
"""
import numpy as np

N_SEQ = 2048
DIM = 1024
H = 16
DH = 64
NC = 8
HPC = 2           # heads per core
CW = HPC * DH     # 128 local head-dim columns
ISL = N_SEQ // NC # 256 output rows per core
LN_EPS = 1e-6
NEG = -1e30

_CACHE = {}


def _av_segments(off):
    """Column segments of a 1024-wide block, split at PSUM bank (512) bounds."""
    if off < 512:
        return [(off, 512), (512, 1024)]
    return [(off, 1024)]


def _build(debug=False):
    import concourse.bass as bass
    import concourse.bacc as bacc
    import concourse.tile as tile
    import concourse.mybir as mybir

    F32 = mybir.dt.float32
    BF = mybir.dt.bfloat16
    AF = mybir.ActivationFunctionType
    ALU = mybir.AluOpType

    nc = bacc.Bacc("TRN2", target_bir_lowering=False, debug=False, num_devices=NC)

    x_d = nc.dram_tensor("x_bf", [N_SEQ, DIM], BF, kind="ExternalInput")
    wblk_d = nc.dram_tensor("w_blk", [DIM, 3 * CW], BF, kind="ExternalInput")
    wout_d = nc.dram_tensor("w_out", [DIM, DIM], BF, kind="ExternalInput")
    qb_d = nc.dram_tensor("qb", [128, 3], F32, kind="ExternalInput")
    cos_d = nc.dram_tensor("cos2t", [CW, N_SEQ], BF, kind="ExternalInput")
    sin_d = nc.dram_tensor("sin2t", [CW, N_SEQ], BF, kind="ExternalInput")
    pb_d = nc.dram_tensor("pb2d", [128, 16], F32, kind="ExternalInput")
    pb01_d = nc.dram_tensor("pb01", [128, 16], BF, kind="ExternalInput")
    tri_d = nc.dram_tensor("tri2", [128, 256], BF, kind="ExternalInput")
    p128_d = nc.dram_tensor("p128", [128, 128], BF, kind="ExternalInput")
    ident_d = nc.dram_tensor("ident", [128, 128], BF, kind="ExternalInput")
    out_d = nc.dram_tensor("out_sl", [ISL, DIM], F32, kind="ExternalOutput")

    groups = [list(range(NC))]
    KC = DIM // 128  # 8 contraction chunks
    NB = N_SEQ // 128  # 16 sequence blocks

    with tile.TileContext(nc) as tc:
        with tc.tile_pool(name="cst", bufs=1) as cst, \
             tc.tile_pool(name="big", bufs=1) as big, \
             tc.tile_pool(name="wrk", bufs=2) as wrk, \
             tc.tile_pool(name="xb", bufs=1) as xbp, \
             tc.tile_pool(name="et", bufs=6) as etp, \
             tc.tile_pool(name="dram", bufs=1, space="DRAM") as drp:

            a2a_in = drp.tile([NC * 128, ISL], BF, tag="a2a_in")
            a2a_out = drp.tile([NC * 128, ISL], BF, tag="a2a_out")
            wup_in = drp.tile([128, 8], BF, tag="wup_in")
            wup_out = drp.tile([NC * 128, 8], BF, tag="wup_out", addr_space="Shared")

            # ---------- warm-up collective enqueued first ----------
            # absorbs the collective-entry barrier during LayerNorm and keeps
            # the ncfw stream stepping so the real A2A later starts fast.
            wup_sb = cst.tile([128, 8], BF, tag="wup_sb")
            nc.vector.memset(wup_sb[:], 0.0)
            nc.sync.dma_start(wup_in[:, :], wup_sb[:])
            nc.gpsimd.collective_compute(
                "AllGather", ALU.bypass, replica_groups=groups,
                ins=[wup_in.opt()], outs=[wup_out.opt()])

            # ---------- inputs: x split across 4 DMA queues, consts behind ----------
            x_all = xbp.tile([128, NB, DIM], BF, tag="x_all")
            xsrc = x_d.ap().rearrange("(b p) d -> p b d", p=128)
            dma_engs = [nc.sync, nc.scalar, nc.gpsimd]
            for b in range(NB):
                dma_engs[b % 3].dma_start(
                    x_all[:, b:b + 1, :], xsrc[:, b:b + 1, :])
            xblk = [x_all[:, b, :] for b in range(NB)]
            ident_t = cst.tile([128, 128], BF, tag="ident")
            nc.sync.dma_start(ident_t[:], ident_d.ap())
            qb_t = cst.tile([128, 3], F32, tag="qb")
            nc.sync.dma_start(qb_t[:], qb_d.ap())
            pb_t = cst.tile([128, 16], F32, tag="pb")
            pb01_t = cst.tile([128, 16], BF, tag="pb01")
            tri_t = cst.tile([128, 256], BF, tag="tri")
            p128_t = cst.tile([128, 128], BF, tag="p128")
            nc.sync.dma_start(pb_t[:], pb_d.ap())
            nc.sync.dma_start(pb01_t[:], pb01_d.ap())
            nc.sync.dma_start(tri_t[:], tri_d.ap())
            nc.sync.dma_start(p128_t[:], p128_d.ap())
            w_all = cst.tile([128, KC, 3 * CW], BF, tag="w_all")
            nc.sync.dma_start(
                w_all[:], wblk_d.ap().rearrange("(kc p) c -> p kc c", p=128))
            w_t = [w_all[:, kc, :] for kc in range(KC)]
            cos_t = cst.tile([CW, N_SEQ], BF, tag="cos")
            sin_t = cst.tile([CW, N_SEQ], BF, tag="sin")
            nc.scalar.dma_start(cos_t[:], cos_d.ap())
            nc.scalar.dma_start(sin_t[:], sin_d.ap())
            wo_all = cst.tile([128, KC, DIM], BF, tag="wo_all")
            nc.sync.dma_start(
                wo_all[:], wout_d.ap().rearrange("(kc p) c -> p kc c", p=128))
            wo_t = [wo_all[:, kc, :] for kc in range(KC)]

            zeps = cst.tile([128, 2], F32, tag="zeps")
            nc.vector.memset(zeps[:, 0:1], 0.0)
            nc.vector.memset(zeps[:, 1:2], LN_EPS)
            ones64 = cst.tile([1, 64], BF, tag="ones64")
            nc.vector.memset(ones64[:], 1.0)
            # av lhsT per j-chunk: [v_h0(64) | 1 | v_h1(64) | 1] -> 130 cols each
            av_all = big.tile([128, 16 * 130], BF, tag="av_all")
            av_v = av_all[:].rearrange("p (jc c) -> p jc c", c=130)
            nc.vector.memset(av_v[:, :, 64:65], 1.0)
            nc.vector.memset(av_v[:, :, 129:130], 1.0)

            psC = tc.tile_pool(name="psC", bufs=1, space="PSUM")
            ps = psC.__enter__()

            # ---------- phase 1: replicated LayerNorm ----------
            # stats on DVE (bn_stats/bn_aggr), batched Rsqrt on scalar,
            # normalize-apply on gpsimd, transpose copies split scalar/DVE.
            mv_all = wrk.tile([128, 2 * NB], F32, tag="mv")
            for b in range(NB):
                st = wrk.tile([128, 12], F32, tag="st6", bufs=4)
                nc.vector.bn_stats(st[:, 0:6], xblk[b][:, 0:512])
                nc.vector.bn_stats(st[:, 6:12], xblk[b][:, 512:1024])
                nc.vector.bn_aggr(mv_all[:, 2 * b:2 * b + 2], st[:])
            rstd_all = wrk.tile([128, NB], F32, tag="rstd")
            nmr_all = wrk.tile([128, NB], F32, tag="nmr")
            for g in range(8):
                gs = slice(g * 2, (g + 1) * 2)
                # rstd = 1/sqrt(var + eps)
                sqv = wrk.tile([128, 2], F32, tag="sqv", bufs=4)
                nc.scalar.activation(
                    sqv[:],
                    mv_all[:].rearrange("p (b tw) -> p b tw", tw=2)[:, gs, 1:2],
                    AF.Sqrt, bias=zeps[:, 1:2])
                nc.vector.reciprocal(rstd_all[:, gs], sqv[:])
                # nmr = -mean * rstd
                nc.vector.scalar_tensor_tensor(
                    nmr_all[:, gs],
                    mv_all[:].rearrange("p (b tw) -> p b tw", tw=2)[:, gs, 0:1],
                    -1.0, rstd_all[:, gs], ALU.mult, ALU.mult)
            xnT_g = []
            for grp in range(2):
                t = big.tile([128, NB * 512], BF, tag=f"xnTg{grp}")
                xnT_g.append(t)
            for b in range(NB):
                xn = wrk.tile([128, DIM], BF, tag="ln_xn", bufs=4)
                nc.gpsimd.tensor_scalar(
                    xn[:], xblk[b],
                    rstd_all[:, b:b + 1], nmr_all[:, b:b + 1],
                    ALU.mult, ALU.add)
                for grp in range(2):
                    tp_f = ps.tile([128, 1024], F32, tag="sim", bufs=2)
                    tp = tp_f[:].bitcast(BF)
                    for q in range(4):
                        kc = grp * 4 + q
                        nc.tensor.transpose(
                            tp[:, q * 128:(q + 1) * 128],
                            xn[:, kc * 128:(kc + 1) * 128], ident_t[:])
                    dst = xnT_g[grp][:, b * 512:(b + 1) * 512]
                    nc.scalar.copy(dst, tp[:, 0:512])

            # ---------- phase 2: qkv^T (weight-stationary halves) + rope ----------
            # Emission order on PE: all QKV matmuls (both halves), then the
            # rotation matmuls (their qT/kT inputs were copied out long
            # before), then V transposes - so the PE never waits on copies.
            qropeT = big.tile([CW, N_SEQ], BF, tag="qropeT")
            krop_h0 = big.tile([CW, N_SEQ], BF, tag="krop_h0")
            krop_h1 = big.tile([CW, N_SEQ], BF, tag="krop_h1")
            nc.vector.memset(krop_h0[64:128, :], 0.0)
            nc.vector.memset(krop_h1[0:64, :], 0.0)
            kroph = [krop_h0, krop_h1]
            vT_sb = big.tile([CW, N_SEQ], BF, tag="vT")
            qT_h = [None, None]
            kT_h = [None, None]

            def emit_qkv(half, cp_vec):
                hc = slice(half * 1024, (half + 1) * 1024)
                ps_q = ps.tile([128, 1024], F32, tag="sim", bufs=2)
                ps_k = ps.tile([128, 1024], F32, tag="sim", bufs=2)
                ps_v = ps.tile([128, 1024], F32, tag="sim", bufs=2)
                for kc in range(KC):
                    st = (kc == 0); sp = (kc == KC - 1)
                    grp, kcq = kc // 4, kc % 4
                    xview = xnT_g[grp][:].rearrange(
                        "p (b q c) -> p b q c", q=4, c=128)
                    for seg in range(2):
                        cs = slice(seg * 512, (seg + 1) * 512)
                        b0 = 8 * half + 4 * seg
                        rhs = xview[:, b0:b0 + 4, kcq, :]
                        nc.tensor.matmul(ps_q[:, cs], w_t[kc][:, 0:128], rhs,
                                         start=st, stop=sp, skip_group_check=True)
                        nc.tensor.matmul(ps_k[:, cs], w_t[kc][:, 128:256], rhs,
                                         start=st, stop=sp, skip_group_check=True)
                        nc.tensor.matmul(ps_v[:, cs], w_t[kc][:, 256:384], rhs,
                                         start=st, stop=sp, skip_group_check=True)
                qT_sb = wrk.tile([128, 1024], BF, tag="qT_sb")
                kT_sb = wrk.tile([128, 1024], BF, tag="kT_sb")
                if cp_vec:
                    nc.vector.tensor_scalar_add(qT_sb[:], ps_q[:], qb_t[:, 0:1])
                    nc.vector.tensor_scalar_add(kT_sb[:], ps_k[:], qb_t[:, 1:2])
                    nc.vector.tensor_scalar_add(vT_sb[:, hc], ps_v[:], qb_t[:, 2:3])
                else:
                    nc.scalar.activation(qT_sb[:], ps_q[:], AF.Identity, bias=qb_t[:, 0:1])
                    nc.scalar.activation(kT_sb[:], ps_k[:], AF.Identity, bias=qb_t[:, 1:2])
                    nc.scalar.activation(vT_sb[:, hc], ps_v[:], AF.Identity, bias=qb_t[:, 2:3])
                qT_h[half] = qT_sb
                kT_h[half] = kT_sb

            def emit_rope(half):
                hc = slice(half * 1024, (half + 1) * 1024)
                ps_qr = ps.tile([128, 1024], F32, tag="sim", bufs=2)
                ps_kr = ps.tile([128, 1024], F32, tag="sim", bufs=2)
                for seg in range(2):
                    cs = slice(seg * 512, (seg + 1) * 512)
                    nc.tensor.matmul(ps_qr[:, cs], p128_t[:], qT_h[half][:, cs],
                                     start=True, stop=True, skip_group_check=True)
                    nc.tensor.matmul(ps_kr[:, cs], p128_t[:], kT_h[half][:, cs],
                                     start=True, stop=True, skip_group_check=True)
                for ki, (src_sb, src_r) in enumerate(((qT_h[half], ps_qr),
                                                      (kT_h[half], ps_kr))):
                    t1 = wrk.tile([128, 1024], BF, tag="rp1")
                    nc.gpsimd.tensor_mul(t1[:], src_sb[:], cos_t[:, hc])
                    t2 = wrk.tile([128, 1024], BF, tag="rp2")
                    nc.vector.scalar_tensor_tensor(
                        t2[:], src_r[:], 1.0, sin_t[:, hc], ALU.mult, ALU.mult)
                    if ki == 0:
                        nc.vector.tensor_add(qropeT[:, hc], t1[:], t2[:])
                    else:
                        nc.vector.tensor_add(
                            krop_h0[0:64, hc], t1[0:64, :], t2[0:64, :])
                        nc.vector.tensor_add(
                            krop_h1[64:128, hc], t1[64:128, :], t2[64:128, :])

            def emit_vt(half):
                for grp in range(2 * half, 2 * half + 2):
                    tp_f = ps.tile([128, 1024], F32, tag="sim", bufs=2)
                    tp = tp_f[:].bitcast(BF)
                    for q in range(4):
                        jc = grp * 4 + q
                        nc.tensor.transpose(
                            tp[:, q * 128:(q + 1) * 128],
                            vT_sb[:, jc * 128:(jc + 1) * 128], ident_t[:])
                    for q in range(4):
                        jc = grp * 4 + q
                        src = tp[:, q * 128:(q + 1) * 128]
                        eng = nc.scalar.copy if q % 2 == 0 else (
                            lambda d, s: nc.vector.tensor_scalar_mul(d, s, 1.0))
                        eng(av_all[:, jc * 130 + 0: jc * 130 + 64], src[:, 0:64])
                        eng(av_all[:, jc * 130 + 65: jc * 130 + 129], src[:, 64:128])

            # ---------- attention + pre-A2A normalize (interleaved with
            # half-1 QKV so the scalar-bound exp stream starts early) ----------
            avh_all = {}

            def emit_sim(ib4, jc):
                i0 = ib4 * 1024
                off = max(0, 128 * jc - i0)
                diag = 128 * jc >= i0
                tsel = 0 if jc == 0 else 128
                ets = []
                for h in range(2):
                    sim = ps.tile([128, 1024], F32, tag="sim", bufs=2)
                    for (a, b) in _av_segments(off):
                        has_diag = diag and a <= off < b
                        nc.tensor.matmul(
                            sim[:, a:b],
                            kroph[h][:, jc * 128:(jc + 1) * 128],
                            qropeT[:, i0 + a:i0 + b],
                            start=True, stop=not has_diag,
                            skip_group_check=True)
                        if has_diag:
                            # causal mask added on the PE itself
                            nc.tensor.matmul(
                                sim[:, off:off + 128],
                                ident_t[:], tri_t[:, tsel:tsel + 128],
                                start=False, stop=True,
                                skip_group_check=True)
                    e_t = etp.tile([128, 1024], BF, tag="e_t")
                    nc.scalar.activation(e_t[:, off:], sim[:, off:], AF.Exp,
                                         bias=pb_t[:, jc:jc + 1])
                    ets.append(e_t)
                return ets

            def emit_av(ib4, jc, ets):
                i0 = ib4 * 1024
                jmax = 8 * ib4 + 7
                off = max(0, 128 * jc - i0)
                for h in range(2):
                    for (a, b) in _av_segments(off):
                        last = (ib4 == 1 and jc == jmax and b == 1024)
                        nc.tensor.matmul(
                            avh_all[(ib4, h)][:, a:b],
                            av_all[:, jc * 130 + 65 * h: jc * 130 + 65 * h + 65],
                            ets[h][:, a:b],
                            start=(jc == 0), stop=last,
                            skip_group_check=True)

            def attn_main(ib4):
                for h in range(2):
                    av_t = ps.tile([65, 1024], F32, tag="av", bufs=2)
                    avh_all[(ib4, h)] = av_t
                jmax = 8 * ib4 + 7
                pend = None
                for jc in range(jmax + 1):
                    ets = emit_sim(ib4, jc)
                    if pend is not None:
                        emit_av(ib4, pend[0], pend[1])
                    pend = (jc, ets)
                emit_av(ib4, pend[0], pend[1])

            def attn_tail(ib4):
                for h in range(2):
                    av = avh_all[(ib4, h)]
                    if ib4 == 0:
                        # column i=0 attends to all j: chunks 1..15 add col 0
                        e0full = ps.tile([128, 1024], F32, tag="sim", bufs=2)
                        e0ps = e0full[:, 0:16]
                        for jc in range(1, 16):
                            nc.tensor.matmul(
                                e0ps[:, jc:jc + 1],
                                kroph[h][:, jc * 128:(jc + 1) * 128],
                                qropeT[:, 0:1],
                                start=(jc == 1), stop=(jc == 15), skip_group_check=True)
                        e0e = wrk.tile([128, 16], BF, tag="e0e")
                        nc.scalar.activation(e0e[:], e0ps[:], AF.Exp, bias=zeps[:, 0:1])
                        e0m = wrk.tile([128, 16], BF, tag="e0m")
                        nc.vector.tensor_mul(e0m[:], e0e[:], pb01_t[:])
                        for jc in range(1, 16):
                            nc.tensor.matmul(
                                av[:, 0:1],
                                av_all[:, jc * 130 + 65 * h: jc * 130 + 65 * h + 65],
                                e0m[:, jc:jc + 1],
                                start=False, stop=(jc == 15), skip_group_check=True)
                    # normalize: avn = av[0:64] / av[64]
                    den = wrk.tile([1, 1024], BF, tag="den")
                    nc.vector.tensor_scalar_mul(den[:], av[64:65, :], 1.0)
                    bps = ps.tile([128, 1024], F32, tag="sim", bufs=2)
                    for seg in range(2):
                        cs = slice(seg * 512, (seg + 1) * 512)
                        nc.tensor.matmul(bps[0:64, cs], ones64[:], den[:, cs],
                                         start=True, stop=True, skip_group_check=True)
                    recb = wrk.tile([64, 1024], F32, tag="recb")
                    nc.vector.reciprocal_approx_fast(recb[:], bps[0:64, :])
                    avs = wrk.tile([64, 1024], BF, tag="avs")
                    nc.vector.tensor_mul(avs[:], av[0:64, :], recb[:])
                    nc.sync.dma_start(
                        a2a_in[:].rearrange("(blk p) i -> p blk i", p=128)
                               [64 * h:64 * h + 64, 4 * ib4:4 * ib4 + 4, :],
                        avs[:].rearrange("p (blk i) -> p blk i", blk=4))

            def emit_qkv1_piece(which):
                # one of q/k/v for half 1, fully accumulated + copied out so
                # the sim-tag rotation never stalls on a half-filled tile
                half = 1
                hc = slice(half * 1024, (half + 1) * 1024)
                pp = ps.tile([128, 1024], F32, tag="sim", bufs=2)
                col = {"q": 0, "k": 1, "v": 2}[which]
                for kc in range(KC):
                    st = (kc == 0); sp = (kc == KC - 1)
                    grp, kcq = kc // 4, kc % 4
                    xview = xnT_g[grp][:].rearrange(
                        "p (b q c) -> p b q c", q=4, c=128)
                    for seg in range(2):
                        cs = slice(seg * 512, (seg + 1) * 512)
                        b0 = 8 * half + 4 * seg
                        rhs = xview[:, b0:b0 + 4, kcq, :]
                        nc.tensor.matmul(
                            pp[:, cs], w_t[kc][:, col * 128:(col + 1) * 128],
                            rhs, start=st, stop=sp, skip_group_check=True)
                if which == "q":
                    qT_sb = wrk.tile([128, 1024], BF, tag="qT_sb")
                    nc.vector.tensor_scalar_add(qT_sb[:], pp[:], qb_t[:, 0:1])
                    qT_h[half] = qT_sb
                elif which == "k":
                    kT_sb = wrk.tile([128, 1024], BF, tag="kT_sb")
                    nc.vector.tensor_scalar_add(kT_sb[:], pp[:], qb_t[:, 1:2])
                    kT_h[half] = kT_sb
                else:
                    nc.vector.tensor_scalar_add(vT_sb[:, hc], pp[:], qb_t[:, 2:3])

            def attn_main0_interleaved():
                ib4 = 0
                for h in range(2):
                    av_t = ps.tile([65, 1024], F32, tag="av", bufs=2)
                    avh_all[(ib4, h)] = av_t
                pend = None
                for jc in range(8):
                    ets = emit_sim(ib4, jc)
                    if pend is not None:
                        emit_av(ib4, pend[0], pend[1])
                    pend = (jc, ets)
                    if jc == 2:
                        emit_qkv1_piece("q")
                    elif jc == 4:
                        emit_qkv1_piece("k")
                    elif jc == 6:
                        emit_qkv1_piece("v")
                emit_av(ib4, pend[0], pend[1])

            emit_qkv(0, cp_vec=False)
            emit_rope(0)
            emit_vt(0)
            attn_main0_interleaved()
            emit_rope(1)
            emit_vt(1)
            attn_tail(0)
            attn_main(1)
            attn_tail(1)

            # ---------- phase 5: A2A reshard heads -> sequence (bf16) ----------
            nc.gpsimd.collective_compute(
                "AllToAll", ALU.bypass, replica_groups=groups,
                ins=[a2a_in.opt()], outs=[a2a_out.opt()])

            psC.__exit__(None, None, None)
            psD = tc.tile_pool(name="psD", bufs=1, space="PSUM")
            ps = psD.__enter__()
            # ---------- keep the PE clock-gate warm through the A2A ----------
            dmy = ps.tile([128, 256], F32, tag="dmy")
            for _ in range(48):
                nc.tensor.matmul(dmy[:], tri_t[:, 0:128], tri_t[:, 0:256],
                                 start=True, stop=True, skip_group_check=True)
            # ---------- phase 6: out-projection ----------
            rcv_all = big.tile([128, NC * ISL], BF, tag="rcv_all")
            rcv_v = rcv_all[:].rearrange("p (blk i) -> p blk i", blk=NC)
            a2o_v = a2a_out[:].rearrange("(blk p) i -> p blk i", p=128)
            nc.sync.dma_start(rcv_v[:, 0:4, :], a2o_v[:, 0:4, :])
            nc.scalar.dma_start(rcv_v[:, 4:8, :], a2o_v[:, 4:8, :])
            for icx in range(2):
                op0 = ps.tile([128, 512], F32, tag="op", bufs=2)
                op1 = ps.tile([128, 512], F32, tag="op", bufs=2)
                for kb in range(NC):
                    st = (kb == 0); sp = (kb == NC - 1)
                    lhs = rcv_all[:, kb * ISL + icx * 128: kb * ISL + (icx + 1) * 128]
                    nc.tensor.matmul(op0[:], lhs, wo_t[kb][:, 0:512], start=st, stop=sp)
                    nc.tensor.matmul(op1[:], lhs, wo_t[kb][:, 512:1024], start=st, stop=sp)
                ob = wrk.tile([128, DIM], F32, tag="ob")
                nc.vector.tensor_scalar_mul(ob[:, 0:512], op0[:], 1.0)
                nc.vector.tensor_scalar_mul(ob[:, 512:1024], op1[:], 1.0)
                nc.sync.dma_start(out_d.ap()[icx * 128:(icx + 1) * 128, :], ob[:])
            psD.__exit__(None, None, None)

    nc.compile()
    return nc


def _host_prep(x, pos_sin, pos_cos, mask, ln_scale, ln_bias, w_qkv, w_out, b_out):
    f32 = np.float32
    import ml_dtypes
    bf16 = ml_dtypes.bfloat16
    scale = np.float32(DIM ** -0.5)
    x = np.asarray(x, f32); pos_sin = np.asarray(pos_sin, f32)
    pos_cos = np.asarray(pos_cos, f32); mask = np.asarray(mask)
    ln_scale = np.asarray(ln_scale, f32); ln_bias = np.asarray(ln_bias, f32)
    w_qkv = np.asarray(w_qkv, f32); w_out = np.asarray(w_out, f32)

    W = w_qkv * ln_scale[:, None]
    qb_full = (ln_bias @ w_qkv).astype(f32)  # [3072]

    x_bf = np.ascontiguousarray(x).astype(bf16)

    cos_full = np.ones((N_SEQ, DH // 2), f32)
    sin_full = np.zeros((N_SEQ, DH // 2), f32)
    cos_full[1:] = pos_cos
    sin_full[1:] = pos_sin
    cos2t = np.ascontiguousarray(np.tile(np.repeat(cos_full, 2, axis=1).T, (2, 1))).astype(bf16)
    sin2t = np.ascontiguousarray(np.tile(np.repeat(sin_full, 2, axis=1).T, (2, 1))).astype(bf16)

    pb_vec = np.zeros(N_SEQ, f32)
    pb_vec[1:] = np.where(mask, 0.0, NEG).astype(f32)
    pb2d = np.ascontiguousarray(pb_vec.reshape(16, 128).T)
    pb01 = np.ascontiguousarray((pb2d == 0)).astype(bf16)

    idg = np.arange(128)
    triu = (idg[None, :] >= idg[:, None])
    tri_first = np.where(triu | (idg[None, :] == 0), 0.0, NEG).astype(f32)
    tri_rest = np.where(triu, 0.0, NEG).astype(f32)
    tri2 = np.ascontiguousarray(np.concatenate([tri_first, tri_rest], axis=1)).astype(bf16)

    p128 = np.zeros((128, 128), f32)
    t = np.arange(64)
    p128[2 * t + 1, 2 * t] = -1.0
    p128[2 * t, 2 * t + 1] = 1.0
    p128 = p128.astype(bf16)

    ident = np.eye(128, dtype=f32).astype(bf16)
    w_out_c = np.ascontiguousarray(w_out).astype(bf16)

    in_maps = []
    for r in range(NC):
        hc = slice(CW * r, CW * (r + 1))
        w_blk = np.ascontiguousarray(np.concatenate(
            [W[:, 0:H * DH][:, hc] * scale,
             W[:, H * DH:2 * H * DH][:, hc],
             W[:, 2 * H * DH:][:, hc]], axis=1)).astype(bf16)
        qb = np.concatenate(
            [qb_full[0:H * DH][hc] * scale,
             qb_full[H * DH:2 * H * DH][hc],
             qb_full[2 * H * DH:][hc]]).astype(f32)
        in_maps.append({
            "x_bf": x_bf,
            "w_blk": w_blk,
            "w_out": w_out_c,
            "qb": np.ascontiguousarray(qb.reshape(3, CW).T),
            "cos2t": cos2t, "sin2t": sin2t,
            "pb2d": pb2d, "pb01": pb01, "tri2": tri2,
            "p128": p128, "ident": ident,
        })
    return in_maps


def _kernel_impl(inputs, trace=False, tmpdir=None):
    from concourse.bass_utils import run_bass_kernel_spmd
    if "nc" not in _CACHE:
        _CACHE["nc"] = _build()
    nc = _CACHE["nc"]
    in_maps = _host_prep(**inputs)
    kwargs = {}
    if trace:
        import sys, types
        try:
            from antenv.axon_hooks import get_axon_ntff_profile_hook  # noqa: F401
        except ImportError:
            from trn_agent_boot.trn_boot import _ntff_profile_via_ctypes
            hook = _ntff_profile_via_ctypes('/opt/axon/libaxon_pjrt.so')
            mod = types.ModuleType('antenv.axon_hooks')
            mod.get_axon_ntff_profile_hook = lambda: hook
            sys.modules['antenv.axon_hooks'] = mod
        kwargs = {"trace": True, "tmpdir": tmpdir}
    res = run_bass_kernel_spmd(nc, in_maps, list(range(NC)), **kwargs)
    out = np.concatenate([res.results[r]["out_sl"] for r in range(NC)], axis=0)
    out = out + np.asarray(inputs["b_out"], np.float32)[None, :]
    return out, res.exec_time_ns


def kernel(**inputs) -> np.ndarray:
    out, _ = _kernel_impl(inputs)
    return out


# revision 39
# speedup vs baseline: 1.1374x; 1.0858x over previous
"""Trainium2 Bass kernel for nn_Attention_6992206758310.

Dense transformer block: LayerNorm -> QKV -> selective RoPE -> head-last
masked attention (softmax over j) -> out-projection.

Sharding: heads (16) are split 2-per-core across 8 NeuronCores (tensor
parallel). LayerNorm is REPLICATED on every core (it is cheap and runs
entirely inside the ~66us collective-entry dead window, eliminating the
AllGather a sharded LayerNorm would need). Each core computes QKV + RoPE
for its 2 heads over the full sequence, runs attention in sim^T [j, i]
layout (softmax over the partition axis becomes a matmul-accumulated
column sum via an appended ones-column on V), normalizes by the softmax
denominator on the head-parallel side, AllToAll-reshards to
sequence-parallel, and projects through w_out so each core emits its own
256-row slice of the output. Host concatenates slices.

All matmuls are bf16 (fp32 PSUM accumulate). The scalar engine runs only
Exp (plus one batched Rsqrt + Identity copies) to avoid activation-table
thrash; other elementwise work is on vector/gpsimd. Dummy fp32 matmuls
during the AllToAll keep the PE clock-gate warm for the out-projection.
"""
import numpy as np

N_SEQ = 2048
DIM = 1024
H = 16
DH = 64
NC = 8
HPC = 2           # heads per core
CW = HPC * DH     # 128 local head-dim columns
ISL = N_SEQ // NC # 256 output rows per core
LN_EPS = 1e-6
NEG = -1e30

_CACHE = {}


def _av_segments(off):
    """Column segments of a 1024-wide block, split at PSUM bank (512) bounds."""
    if off < 512:
        return [(off, 512), (512, 1024)]
    return [(off, 1024)]


def _build(debug=False):
    import concourse.bass as bass
    import concourse.bacc as bacc
    import concourse.tile as tile
    import concourse.mybir as mybir

    F32 = mybir.dt.float32
    BF = mybir.dt.bfloat16
    AF = mybir.ActivationFunctionType
    ALU = mybir.AluOpType

    nc = bacc.Bacc("TRN2", target_bir_lowering=False, debug=False, num_devices=NC)

    x_d = nc.dram_tensor("x_bf", [N_SEQ, DIM], BF, kind="ExternalInput")
    wblk_d = nc.dram_tensor("w_blk", [DIM, 3 * CW], BF, kind="ExternalInput")
    wout_d = nc.dram_tensor("w_out", [DIM, DIM], BF, kind="ExternalInput")
    qb_d = nc.dram_tensor("qb", [128, 3], F32, kind="ExternalInput")
    cos_d = nc.dram_tensor("cos2t", [CW, N_SEQ], BF, kind="ExternalInput")
    sin_d = nc.dram_tensor("sin2t", [CW, N_SEQ], BF, kind="ExternalInput")
    pb_d = nc.dram_tensor("pb2d", [128, 16], F32, kind="ExternalInput")
    pb01_d = nc.dram_tensor("pb01", [128, 16], BF, kind="ExternalInput")
    tri_d = nc.dram_tensor("tri2", [128, 256], BF, kind="ExternalInput")
    p128_d = nc.dram_tensor("p128", [128, 128], BF, kind="ExternalInput")
    ident_d = nc.dram_tensor("ident", [128, 128], BF, kind="ExternalInput")
    out_d = nc.dram_tensor("out_sl", [ISL, DIM], F32, kind="ExternalOutput")

    groups = [list(range(NC))]
    KC = DIM // 128  # 8 contraction chunks
    NB = N_SEQ // 128  # 16 sequence blocks

    with tile.TileContext(nc) as tc:
        with tc.tile_pool(name="cst", bufs=1) as cst, \
             tc.tile_pool(name="big", bufs=1) as big, \
             tc.tile_pool(name="wrk", bufs=2) as wrk, \
             tc.tile_pool(name="xb", bufs=1) as xbp, \
             tc.tile_pool(name="et", bufs=6) as etp, \
             tc.tile_pool(name="dram", bufs=1, space="DRAM") as drp:

            a2a_in = drp.tile([NC * 128, ISL], BF, tag="a2a_in")
            a2a_out = drp.tile([NC * 128, ISL], BF, tag="a2a_out")
            wup_in = drp.tile([128, 8], BF, tag="wup_in")
            wup_out = drp.tile([NC * 128, 8], BF, tag="wup_out", addr_space="Shared")

            # ---------- warm-up collective enqueued first ----------
            # absorbs the collective-entry barrier during LayerNorm and keeps
            # the ncfw stream stepping so the real A2A later starts fast.
            wup_sb = cst.tile([128, 8], BF, tag="wup_sb")
            nc.vector.memset(wup_sb[:], 0.0)
            nc.sync.dma_start(wup_in[:, :], wup_sb[:])
            nc.gpsimd.collective_compute(
                "AllGather", ALU.bypass, replica_groups=groups,
                ins=[wup_in.opt()], outs=[wup_out.opt()])

            # ---------- inputs: x split across 4 DMA queues, consts behind ----------
            x_all = xbp.tile([128, NB, DIM], BF, tag="x_all")
            xsrc = x_d.ap().rearrange("(b p) d -> p b d", p=128)
            dma_engs = [nc.sync, nc.scalar, nc.gpsimd]
            for b in range(NB):
                dma_engs[b % 3].dma_start(
                    x_all[:, b:b + 1, :], xsrc[:, b:b + 1, :])
            xblk = [x_all[:, b, :] for b in range(NB)]
            ident_t = cst.tile([128, 128], BF, tag="ident")
            nc.sync.dma_start(ident_t[:], ident_d.ap())
            qb_t = cst.tile([128, 3], F32, tag="qb")
            nc.sync.dma_start(qb_t[:], qb_d.ap())
            pb_t = cst.tile([128, 16], F32, tag="pb")
            pb01_t = cst.tile([128, 16], BF, tag="pb01")
            tri_t = cst.tile([128, 256], BF, tag="tri")
            p128_t = cst.tile([128, 128], BF, tag="p128")
            nc.sync.dma_start(pb_t[:], pb_d.ap())
            nc.sync.dma_start(pb01_t[:], pb01_d.ap())
            nc.sync.dma_start(tri_t[:], tri_d.ap())
            nc.sync.dma_start(p128_t[:], p128_d.ap())
            w_all = cst.tile([128, KC, 3 * CW], BF, tag="w_all")
            nc.sync.dma_start(
                w_all[:], wblk_d.ap().rearrange("(kc p) c -> p kc c", p=128))
            w_t = [w_all[:, kc, :] for kc in range(KC)]
            cos_t = cst.tile([CW, N_SEQ], BF, tag="cos")
            sin_t = cst.tile([CW, N_SEQ], BF, tag="sin")
            nc.scalar.dma_start(cos_t[:], cos_d.ap())
            nc.scalar.dma_start(sin_t[:], sin_d.ap())
            wo_all = cst.tile([128, KC, DIM], BF, tag="wo_all")
            nc.sync.dma_start(
                wo_all[:], wout_d.ap().rearrange("(kc p) c -> p kc c", p=128))
            wo_t = [wo_all[:, kc, :] for kc in range(KC)]

            zeps = cst.tile([128, 2], F32, tag="zeps")
            nc.vector.memset(zeps[:, 0:1], 0.0)
            nc.vector.memset(zeps[:, 1:2], LN_EPS)
            ones64 = cst.tile([1, 64], BF, tag="ones64")
            nc.vector.memset(ones64[:], 1.0)
            # av lhsT per j-chunk: [v_h0(64) | 1 | v_h1(64) | 1] -> 130 cols each
            av_all = big.tile([128, 16 * 130], BF, tag="av_all")
            av_v = av_all[:].rearrange("p (jc c) -> p jc c", c=130)
            nc.vector.memset(av_v[:, :, 64:65], 1.0)
            nc.vector.memset(av_v[:, :, 129:130], 1.0)

            psC = tc.tile_pool(name="psC", bufs=1, space="PSUM")
            ps = psC.__enter__()

            # ---------- phase 1: replicated LayerNorm ----------
            # stats on DVE (bn_stats/bn_aggr), batched Rsqrt on scalar,
            # normalize-apply on gpsimd, transpose copies split scalar/DVE.
            mv_all = wrk.tile([128, 2 * NB], F32, tag="mv")
            for b in range(NB):
                st = wrk.tile([128, 12], F32, tag="st6", bufs=4)
                nc.vector.bn_stats(st[:, 0:6], xblk[b][:, 0:512])
                nc.vector.bn_stats(st[:, 6:12], xblk[b][:, 512:1024])
                nc.vector.bn_aggr(mv_all[:, 2 * b:2 * b + 2], st[:])
            rstd_all = wrk.tile([128, NB], F32, tag="rstd")
            nmr_all = wrk.tile([128, NB], F32, tag="nmr")
            for g in range(8):
                gs = slice(g * 2, (g + 1) * 2)
                # rstd = 1/sqrt(var + eps)
                sqv = wrk.tile([128, 2], F32, tag="sqv", bufs=4)
                nc.scalar.activation(
                    sqv[:],
                    mv_all[:].rearrange("p (b tw) -> p b tw", tw=2)[:, gs, 1:2],
                    AF.Sqrt, bias=zeps[:, 1:2])
                nc.vector.reciprocal(rstd_all[:, gs], sqv[:])
                # nmr = -mean * rstd
                nc.vector.scalar_tensor_tensor(
                    nmr_all[:, gs],
                    mv_all[:].rearrange("p (b tw) -> p b tw", tw=2)[:, gs, 0:1],
                    -1.0, rstd_all[:, gs], ALU.mult, ALU.mult)
            xnT_g = []
            for grp in range(2):
                t = big.tile([128, NB * 512], BF, tag=f"xnTg{grp}")
                xnT_g.append(t)
            for b in range(NB):
                xn = wrk.tile([128, DIM], BF, tag="ln_xn", bufs=4)
                eng = nc.gpsimd if b < 10 else nc.vector
                eng.tensor_scalar(
                    xn[:], xblk[b],
                    rstd_all[:, b:b + 1], nmr_all[:, b:b + 1],
                    ALU.mult, ALU.add)
                for grp in range(2):
                    tp_f = ps.tile([128, 1024], F32, tag="sim", bufs=2)
                    tp = tp_f[:].bitcast(BF)
                    for q in range(4):
                        kc = grp * 4 + q
                        nc.tensor.transpose(
                            tp[:, q * 128:(q + 1) * 128],
                            xn[:, kc * 128:(kc + 1) * 128], ident_t[:])
                    dst = xnT_g[grp][:, b * 512:(b + 1) * 512]
                    nc.scalar.copy(dst, tp[:, 0:512])

            # ---------- phase 2: qkv^T (weight-stationary halves) + rope ----------
            # Emission order on PE: all QKV matmuls (both halves), then the
            # rotation matmuls (their qT/kT inputs were copied out long
            # before), then V transposes - so the PE never waits on copies.
            qropeT = big.tile([CW, N_SEQ], BF, tag="qropeT")
            krop_h0 = big.tile([CW, N_SEQ], BF, tag="krop_h0")
            krop_h1 = big.tile([CW, N_SEQ], BF, tag="krop_h1")
            nc.vector.memset(krop_h0[64:128, :], 0.0)
            nc.vector.memset(krop_h1[0:64, :], 0.0)
            kroph = [krop_h0, krop_h1]
            vT_sb = big.tile([CW, N_SEQ], BF, tag="vT")
            qT_h = [None, None]
            kT_h = [None, None]

            def emit_qkv(half, cp_vec):
                hc = slice(half * 1024, (half + 1) * 1024)
                ps_q = ps.tile([128, 1024], F32, tag="sim", bufs=2)
                ps_k = ps.tile([128, 1024], F32, tag="sim", bufs=2)
                ps_v = ps.tile([128, 1024], F32, tag="sim", bufs=2)
                for kc in range(KC):
                    st = (kc == 0); sp = (kc == KC - 1)
                    grp, kcq = kc // 4, kc % 4
                    xview = xnT_g[grp][:].rearrange(
                        "p (b q c) -> p b q c", q=4, c=128)
                    for seg in range(2):
                        cs = slice(seg * 512, (seg + 1) * 512)
                        b0 = 8 * half + 4 * seg
                        rhs = xview[:, b0:b0 + 4, kcq, :]
                        nc.tensor.matmul(ps_q[:, cs], w_t[kc][:, 0:128], rhs,
                                         start=st, stop=sp, skip_group_check=True)
                        nc.tensor.matmul(ps_k[:, cs], w_t[kc][:, 128:256], rhs,
                                         start=st, stop=sp, skip_group_check=True)
                        nc.tensor.matmul(ps_v[:, cs], w_t[kc][:, 256:384], rhs,
                                         start=st, stop=sp, skip_group_check=True)
                qT_sb = wrk.tile([128, 1024], BF, tag="qT_sb")
                kT_sb = wrk.tile([128, 1024], BF, tag="kT_sb")
                if cp_vec:
                    nc.vector.tensor_scalar_add(qT_sb[:], ps_q[:], qb_t[:, 0:1])
                    nc.vector.tensor_scalar_add(kT_sb[:], ps_k[:], qb_t[:, 1:2])
                    nc.vector.tensor_scalar_add(vT_sb[:, hc], ps_v[:], qb_t[:, 2:3])
                else:
                    nc.scalar.activation(qT_sb[:], ps_q[:], AF.Identity, bias=qb_t[:, 0:1])
                    nc.scalar.activation(kT_sb[:], ps_k[:], AF.Identity, bias=qb_t[:, 1:2])
                    nc.scalar.activation(vT_sb[:, hc], ps_v[:], AF.Identity, bias=qb_t[:, 2:3])
                qT_h[half] = qT_sb
                kT_h[half] = kT_sb

            def emit_rope(half):
                hc = slice(half * 1024, (half + 1) * 1024)
                ps_qr = ps.tile([128, 1024], F32, tag="sim", bufs=2)
                ps_kr = ps.tile([128, 1024], F32, tag="sim", bufs=2)
                for seg in range(2):
                    cs = slice(seg * 512, (seg + 1) * 512)
                    nc.tensor.matmul(ps_qr[:, cs], p128_t[:], qT_h[half][:, cs],
                                     start=True, stop=True, skip_group_check=True)
                    nc.tensor.matmul(ps_kr[:, cs], p128_t[:], kT_h[half][:, cs],
                                     start=True, stop=True, skip_group_check=True)
                for ki, (src_sb, src_r) in enumerate(((qT_h[half], ps_qr),
                                                      (kT_h[half], ps_kr))):
                    t1 = wrk.tile([128, 1024], BF, tag="rp1")
                    nc.gpsimd.tensor_mul(t1[:], src_sb[:], cos_t[:, hc])
                    t2 = wrk.tile([128, 1024], BF, tag="rp2")
                    nc.vector.scalar_tensor_tensor(
                        t2[:], src_r[:], 1.0, sin_t[:, hc], ALU.mult, ALU.mult)
                    if ki == 0:
                        nc.vector.tensor_add(qropeT[:, hc], t1[:], t2[:])
                    else:
                        nc.vector.tensor_add(
                            krop_h0[0:64, hc], t1[0:64, :], t2[0:64, :])
                        nc.vector.tensor_add(
                            krop_h1[64:128, hc], t1[64:128, :], t2[64:128, :])

            def emit_vt(half):
                for grp in range(2 * half, 2 * half + 2):
                    tp_f = ps.tile([128, 1024], F32, tag="sim", bufs=2)
                    tp = tp_f[:].bitcast(BF)
                    for q in range(4):
                        jc = grp * 4 + q
                        nc.tensor.transpose(
                            tp[:, q * 128:(q + 1) * 128],
                            vT_sb[:, jc * 128:(jc + 1) * 128], ident_t[:])
                    for q in range(4):
                        jc = grp * 4 + q
                        src = tp[:, q * 128:(q + 1) * 128]
                        eng = nc.scalar.copy if q % 2 == 0 else (
                            lambda d, s: nc.vector.tensor_scalar_mul(d, s, 1.0))
                        eng(av_all[:, jc * 130 + 0: jc * 130 + 64], src[:, 0:64])
                        eng(av_all[:, jc * 130 + 65: jc * 130 + 129], src[:, 64:128])

            # ---------- attention + pre-A2A normalize (interleaved with
            # half-1 QKV so the scalar-bound exp stream starts early) ----------
            avh_all = {}

            def emit_sim(ib4, jc):
                i0 = ib4 * 1024
                off = max(0, 128 * jc - i0)
                diag = 128 * jc >= i0
                tsel = 0 if jc == 0 else 128
                ets = []
                for h in range(2):
                    sim = ps.tile([128, 1024], F32, tag="sim", bufs=2)
                    for (a, b) in _av_segments(off):
                        has_diag = diag and a <= off < b
                        nc.tensor.matmul(
                            sim[:, a:b],
                            kroph[h][:, jc * 128:(jc + 1) * 128],
                            qropeT[:, i0 + a:i0 + b],
                            start=True, stop=not has_diag,
                            skip_group_check=True)
                        if has_diag:
                            # causal mask added on the PE itself
                            nc.tensor.matmul(
                                sim[:, off:off + 128],
                                ident_t[:], tri_t[:, tsel:tsel + 128],
                                start=False, stop=True,
                                skip_group_check=True)
                    e_t = etp.tile([128, 1024], BF, tag="e_t")
                    nc.scalar.activation(e_t[:, off:], sim[:, off:], AF.Exp,
                                         bias=pb_t[:, jc:jc + 1])
                    ets.append(e_t)
                return ets

            def emit_av(ib4, jc, ets):
                i0 = ib4 * 1024
                jmax = 8 * ib4 + 7
                off = max(0, 128 * jc - i0)
                for h in range(2):
                    for (a, b) in _av_segments(off):
                        last = (ib4 == 1 and jc == jmax and b == 1024)
                        nc.tensor.matmul(
                            avh_all[(ib4, h)][:, a:b],
                            av_all[:, jc * 130 + 65 * h: jc * 130 + 65 * h + 65],
                            ets[h][:, a:b],
                            start=(jc == 0), stop=last,
                            skip_group_check=True)

            def attn_main(ib4):
                for h in range(2):
                    av_t = ps.tile([65, 1024], F32, tag="av", bufs=2)
                    avh_all[(ib4, h)] = av_t
                jmax = 8 * ib4 + 7
                pend = None
                for jc in range(jmax + 1):
                    ets = emit_sim(ib4, jc)
                    if pend is not None:
                        emit_av(ib4, pend[0], pend[1])
                    pend = (jc, ets)
                emit_av(ib4, pend[0], pend[1])

            def attn_tail(ib4):
                for h in range(2):
                    av = avh_all[(ib4, h)]
                    if ib4 == 0:
                        # column i=0 attends to all j: chunks 1..15 add col 0
                        e0full = ps.tile([128, 1024], F32, tag="sim", bufs=2)
                        e0ps = e0full[:, 0:16]
                        for jc in range(1, 16):
                            nc.tensor.matmul(
                                e0ps[:, jc:jc + 1],
                                kroph[h][:, jc * 128:(jc + 1) * 128],
                                qropeT[:, 0:1],
                                start=(jc == 1), stop=(jc == 15), skip_group_check=True)
                        e0e = wrk.tile([128, 16], BF, tag="e0e")
                        nc.scalar.activation(e0e[:], e0ps[:], AF.Exp, bias=zeps[:, 0:1])
                        e0m = wrk.tile([128, 16], BF, tag="e0m")
                        nc.vector.tensor_mul(e0m[:], e0e[:], pb01_t[:])
                        for jc in range(1, 16):
                            nc.tensor.matmul(
                                av[:, 0:1],
                                av_all[:, jc * 130 + 65 * h: jc * 130 + 65 * h + 65],
                                e0m[:, jc:jc + 1],
                                start=False, stop=(jc == 15), skip_group_check=True)
                    # normalize: avn = av[0:64] / av[64]
                    den = wrk.tile([1, 1024], BF, tag="den")
                    nc.vector.tensor_scalar_mul(den[:], av[64:65, :], 1.0)
                    bps = ps.tile([128, 1024], F32, tag="sim", bufs=2)
                    for seg in range(2):
                        cs = slice(seg * 512, (seg + 1) * 512)
                        nc.tensor.matmul(bps[0:64, cs], ones64[:], den[:, cs],
                                         start=True, stop=True, skip_group_check=True)
                    recb = wrk.tile([64, 1024], F32, tag="recb")
                    nc.vector.reciprocal_approx_fast(recb[:], bps[0:64, :])
                    avs = wrk.tile([64, 1024], BF, tag="avs")
                    nc.vector.tensor_mul(avs[:], av[0:64, :], recb[:])
                    nc.sync.dma_start(
                        a2a_in[:].rearrange("(blk p) i -> p blk i", p=128)
                               [64 * h:64 * h + 64, 4 * ib4:4 * ib4 + 4, :],
                        avs[:].rearrange("p (blk i) -> p blk i", blk=4))

            def emit_qkv1_piece(which):
                # one of q/k/v for half 1, fully accumulated + copied out so
                # the sim-tag rotation never stalls on a half-filled tile
                half = 1
                hc = slice(half * 1024, (half + 1) * 1024)
                pp = ps.tile([128, 1024], F32, tag="sim", bufs=2)
                col = {"q": 0, "k": 1, "v": 2}[which]
                for kc in range(KC):
                    st = (kc == 0); sp = (kc == KC - 1)
                    grp, kcq = kc // 4, kc % 4
                    xview = xnT_g[grp][:].rearrange(
                        "p (b q c) -> p b q c", q=4, c=128)
                    for seg in range(2):
                        cs = slice(seg * 512, (seg + 1) * 512)
                        b0 = 8 * half + 4 * seg
                        rhs = xview[:, b0:b0 + 4, kcq, :]
                        nc.tensor.matmul(
                            pp[:, cs], w_t[kc][:, col * 128:(col + 1) * 128],
                            rhs, start=st, stop=sp, skip_group_check=True)
                if which == "q":
                    qT_sb = wrk.tile([128, 1024], BF, tag="qT_sb")
                    nc.vector.tensor_scalar_add(qT_sb[:], pp[:], qb_t[:, 0:1])
                    qT_h[half] = qT_sb
                elif which == "k":
                    kT_sb = wrk.tile([128, 1024], BF, tag="kT_sb")
                    nc.vector.tensor_scalar_add(kT_sb[:], pp[:], qb_t[:, 1:2])
                    kT_h[half] = kT_sb
                else:
                    nc.vector.tensor_scalar_add(vT_sb[:, hc], pp[:], qb_t[:, 2:3])

            def attn_main0_interleaved():
                ib4 = 0
                for h in range(2):
                    av_t = ps.tile([65, 1024], F32, tag="av", bufs=2)
                    avh_all[(ib4, h)] = av_t
                pend = None
                for jc in range(8):
                    ets = emit_sim(ib4, jc)
                    if pend is not None:
                        emit_av(ib4, pend[0], pend[1])
                    pend = (jc, ets)
                    if jc == 2:
                        emit_qkv1_piece("q")
                    elif jc == 4:
                        emit_qkv1_piece("k")
                    elif jc == 6:
                        emit_qkv1_piece("v")
                emit_av(ib4, pend[0], pend[1])

            def warm(n):
                dmw = ps.tile([128, 1024], F32, tag="sim", bufs=2)
                for _ in range(n):
                    nc.tensor.matmul(dmw[:, 0:512], w_t[0][:, 0:128],
                                     xnT_g[0][:, 0:512],
                                     start=True, stop=True,
                                     skip_group_check=True)

            emit_qkv(0, cp_vec=False)
            emit_rope(0)
            emit_vt(0)
            attn_main0_interleaved()
            emit_rope(1)
            warm(3)
            emit_vt(1)
            attn_tail(0)
            warm(3)
            attn_main(1)
            attn_tail(1)

            # ---------- phase 5: A2A reshard heads -> sequence (bf16) ----------
            nc.gpsimd.collective_compute(
                "AllToAll", ALU.bypass, replica_groups=groups,
                ins=[a2a_in.opt()], outs=[a2a_out.opt()])

            psC.__exit__(None, None, None)
            psD = tc.tile_pool(name="psD", bufs=1, space="PSUM")
            ps = psD.__enter__()
            # ---------- keep the PE clock-gate warm through the A2A ----------
            dmy = ps.tile([128, 256], F32, tag="dmy")
            for _ in range(48):
                nc.tensor.matmul(dmy[:], tri_t[:, 0:128], tri_t[:, 0:256],
                                 start=True, stop=True, skip_group_check=True)
            # ---------- phase 6: out-projection ----------
            rcv_all = big.tile([128, NC * ISL], BF, tag="rcv_all")
            rcv_v = rcv_all[:].rearrange("p (blk i) -> p blk i", blk=NC)
            a2o_v = a2a_out[:].rearrange("(blk p) i -> p blk i", p=128)
            nc.sync.dma_start(rcv_v[:, 0:4, :], a2o_v[:, 0:4, :])
            nc.scalar.dma_start(rcv_v[:, 4:8, :], a2o_v[:, 4:8, :])
            for icx in range(2):
                op0 = ps.tile([128, 512], F32, tag="op", bufs=2)
                op1 = ps.tile([128, 512], F32, tag="op", bufs=2)
                for kb in range(NC):
                    st = (kb == 0); sp = (kb == NC - 1)
                    lhs = rcv_all[:, kb * ISL + icx * 128: kb * ISL + (icx + 1) * 128]
                    nc.tensor.matmul(op0[:], lhs, wo_t[kb][:, 0:512], start=st, stop=sp)
                    nc.tensor.matmul(op1[:], lhs, wo_t[kb][:, 512:1024], start=st, stop=sp)
                ob = wrk.tile([128, DIM], F32, tag="ob")
                nc.vector.tensor_scalar_mul(ob[:, 0:512], op0[:], 1.0)
                nc.vector.tensor_scalar_mul(ob[:, 512:1024], op1[:], 1.0)
                nc.sync.dma_start(out_d.ap()[icx * 128:(icx + 1) * 128, :], ob[:])
            psD.__exit__(None, None, None)

    nc.compile()
    return nc


def _host_prep(x, pos_sin, pos_cos, mask, ln_scale, ln_bias, w_qkv, w_out, b_out):
    f32 = np.float32
    import ml_dtypes
    bf16 = ml_dtypes.bfloat16
    scale = np.float32(DIM ** -0.5)
    x = np.asarray(x, f32); pos_sin = np.asarray(pos_sin, f32)
    pos_cos = np.asarray(pos_cos, f32); mask = np.asarray(mask)
    ln_scale = np.asarray(ln_scale, f32); ln_bias = np.asarray(ln_bias, f32)
    w_qkv = np.asarray(w_qkv, f32); w_out = np.asarray(w_out, f32)

    W = w_qkv * ln_scale[:, None]
    qb_full = (ln_bias @ w_qkv).astype(f32)  # [3072]

    x_bf = np.ascontiguousarray(x).astype(bf16)

    cos_full = np.ones((N_SEQ, DH // 2), f32)
    sin_full = np.zeros((N_SEQ, DH // 2), f32)
    cos_full[1:] = pos_cos
    sin_full[1:] = pos_sin
    cos2t = np.ascontiguousarray(np.tile(np.repeat(cos_full, 2, axis=1).T, (2, 1))).astype(bf16)
    sin2t = np.ascontiguousarray(np.tile(np.repeat(sin_full, 2, axis=1).T, (2, 1))).astype(bf16)

    pb_vec = np.zeros(N_SEQ, f32)
    pb_vec[1:] = np.where(mask, 0.0, NEG).astype(f32)
    pb2d = np.ascontiguousarray(pb_vec.reshape(16, 128).T)
    pb01 = np.ascontiguousarray((pb2d == 0)).astype(bf16)

    idg = np.arange(128)
    triu = (idg[None, :] >= idg[:, None])
    tri_first = np.where(triu | (idg[None, :] == 0), 0.0, NEG).astype(f32)
    tri_rest = np.where(triu, 0.0, NEG).astype(f32)
    tri2 = np.ascontiguousarray(np.concatenate([tri_first, tri_rest], axis=1)).astype(bf16)

    p128 = np.zeros((128, 128), f32)
    t = np.arange(64)
    p128[2 * t + 1, 2 * t] = -1.0
    p128[2 * t, 2 * t + 1] = 1.0
    p128 = p128.astype(bf16)

    ident = np.eye(128, dtype=f32).astype(bf16)
    w_out_c = np.ascontiguousarray(w_out).astype(bf16)

    in_maps = []
    for r in range(NC):
        hc = slice(CW * r, CW * (r + 1))
        w_blk = np.ascontiguousarray(np.concatenate(
            [W[:, 0:H * DH][:, hc] * scale,
             W[:, H * DH:2 * H * DH][:, hc],
             W[:, 2 * H * DH:][:, hc]], axis=1)).astype(bf16)
        qb = np.concatenate(
            [qb_full[0:H * DH][hc] * scale,
             qb_full[H * DH:2 * H * DH][hc],
             qb_full[2 * H * DH:][hc]]).astype(f32)
        in_maps.append({
            "x_bf": x_bf,
            "w_blk": w_blk,
            "w_out": w_out_c,
            "qb": np.ascontiguousarray(qb.reshape(3, CW).T),
            "cos2t": cos2t, "sin2t": sin2t,
            "pb2d": pb2d, "pb01": pb01, "tri2": tri2,
            "p128": p128, "ident": ident,
        })
    return in_maps


def _kernel_impl(inputs, trace=False, tmpdir=None):
    from concourse.bass_utils import run_bass_kernel_spmd
    if "nc" not in _CACHE:
        _CACHE["nc"] = _build()
    nc = _CACHE["nc"]
    in_maps = _host_prep(**inputs)
    kwargs = {}
    if trace:
        import sys, types
        try:
            from antenv.axon_hooks import get_axon_ntff_profile_hook  # noqa: F401
        except ImportError:
            from trn_agent_boot.trn_boot import _ntff_profile_via_ctypes
            hook = _ntff_profile_via_ctypes('/opt/axon/libaxon_pjrt.so')
            mod = types.ModuleType('antenv.axon_hooks')
            mod.get_axon_ntff_profile_hook = lambda: hook
            sys.modules['antenv.axon_hooks'] = mod
        kwargs = {"trace": True, "tmpdir": tmpdir}
    res = run_bass_kernel_spmd(nc, in_maps, list(range(NC)), **kwargs)
    out = np.concatenate([res.results[r]["out_sl"] for r in range(NC)], axis=0)
    out = out + np.asarray(inputs["b_out"], np.float32)[None, :]
    return out, res.exec_time_ns


def kernel(**inputs) -> np.ndarray:
    out, _ = _kernel_impl(inputs)
    return out
